# revision 1
# baseline (speedup 1.0000x reference)
"""Trainium2 Bass kernel for nn_Attention_9612136808713.

Transformer-XL style attention (rel-shift pos bias, causal, 16 heads),
b=2, n=2048, dim=1024. Sharded over 8 NeuronCores: data-parallel on
batch (2) x tensor-parallel on heads (4 groups of 4 heads). Wq/Wkv
column-split per head group; Wo row-split with the partial-sum
reduction done on the host during unsharding.

Self-contained: only needs numpy + the concourse/bass toolchain that is
installed in the environment.
"""

import contextlib
import json

import numpy as np

import concourse.bass as bass
import concourse.mybir as mybir
import concourse.tile as tile
from concourse.bass_utils import run_bass_kernel_spmd

F32 = mybir.dt.float32
F32R = mybir.dt.float32r
BF16 = mybir.dt.float16  # fp16: 10-bit mantissa, values here are bounded

N = 2048
DIM = 1024
HEADS = 16
D = 64          # head dim
HPC = 4         # heads per core
PAIRS = 2       # head pairs per core
CH = 512        # free-dim chunk (one PSUM bank of fp32)
NB = N // 128   # 16 row blocks
KC = DIM // 128  # 8 contraction chunks
SCALE = D ** -0.5
NEG = -30000.0  # exp(-30000) == 0 in fp32, no inf/nan hazards


# --------------------------------------------------------------------------
# Wait-splitting post-pass: this container's walrus build accepts only ONE
# sync-wait command per instruction, while Tile attaches several. Splitting
# an AND-wait into single-wait NoOps on the same engine immediately before
# the instruction is semantically equivalent (sem-ge waits are monotonic).
# --------------------------------------------------------------------------

def _split_waits_json_bytes(raw: bytes) -> bytes:
    d = json.loads(raw)
    counter = [0]

    def fix_block(b):
        out = []
        for inst in b.get("instructions", []):
            si = inst.get("sync_info")
            waits = (si or {}).get("on_wait") or []
            if len(waits) > 1:
                eng = inst.get("engine")
                for w in waits[:-1]:
                    counter[0] += 1
                    out.append(
                        {
                            "engine": eng,
                            "ins": [],
                            "outs": [],
                            "name": f"WSPLIT-{counter[0]}",
                            "opcode": "NoOp",
                            "sync_info": {"on_update": [], "on_wait": [w]},
                        }
                    )
                si["on_wait"] = [waits[-1]]
            out.append(inst)
        b["instructions"] = out

    for f in d.get("functions", []):
        for b in f.get("blocks", []):
            fix_block(b)
    return json.dumps(d).encode()


def _patch_bass(nc):
    orig = nc.to_json_bytes

    def patched():
        return _split_waits_json_bytes(orig())

    nc.to_json_bytes = patched
    return nc


def _span_chunks(I):
    return -(-(128 * (I + 1)) // CH)  # ceil


def build_nc(phases=frozenset({"B", "C", "D"})):
    nc = bass.Bass()

    xT = nc.dram_tensor("xT", [DIM, N], F32R, kind="ExternalInput")
    posT = nc.dram_tensor("posT", [DIM, N], F32R, kind="ExternalInput")
    wq = nc.dram_tensor("wq", [DIM, 256], F32R, kind="ExternalInput")
    wk = nc.dram_tensor("wk", [DIM, 256], F32R, kind="ExternalInput")
    wv = nc.dram_tensor("wv", [DIM, 256], F32R, kind="ExternalInput")
    wp = nc.dram_tensor("wp", [DIM, D], F32R, kind="ExternalInput")
    wo = nc.dram_tensor("wo", [256, DIM], F32R, kind="ExternalInput")
    bq = nc.dram_tensor("bq", [256, 1], F32, kind="ExternalInput")
    bks = nc.dram_tensor("bks", [256, 1], F32, kind="ExternalInput")  # 0.125*bk
    bvb = nc.dram_tensor("bvb", [128, 256], F32, kind="ExternalInput")
    bps = nc.dram_tensor("bps", [D, 1], F32, kind="ExternalInput")  # 0.125*bp
    ident = nc.dram_tensor("ident", [128, 128], BF16, kind="ExternalInput")
    masks = nc.dram_tensor("masks", [128, 4, CH], BF16, kind="ExternalInput")
    out = nc.dram_tensor("out", [N, DIM], BF16, kind="ExternalOutput")

    # pos-bias scratch, one [N, N] bf16 buffer per head
    UB = nc.dram_tensor("UB", [HPC, N * N], BF16)

    with tile.TileContext(nc) as tc:
        with contextlib.ExitStack() as ctx:
            const = ctx.enter_context(tc.tile_pool(name="const", bufs=1))
            pers = ctx.enter_context(tc.tile_pool(name="pers", bufs=1))

            # ---- constants -------------------------------------------------
            ident_sb = const.tile([128, 128], BF16, tag="ident")
            nc.sync.dma_start(out=ident_sb, in_=ident[:, :])
            ident32 = nc.dram_tensor("ident32", [128, 128], F32, kind="ExternalInput")
            ident32_sb = const.tile([128, 128], F32, tag="ident32")
            nc.sync.dma_start(out=ident32_sb, in_=ident32[:, :])
            pattern2 = nc.dram_tensor("pattern2", [2, 128], F32R, kind="ExternalInput")
            pattern2_sb = const.tile([2, 128], F32R, tag="pattern2")
            nc.sync.dma_start(out=pattern2_sb, in_=pattern2[:, :])
            masks_sb = const.tile([128, 4, CH], BF16, tag="masks")
            nc.sync.dma_start(out=masks_sb, in_=masks[:, :, :])
            neg_sb = const.tile([128, 128], BF16, tag="negs")
            nc.vector.memset(neg_sb, NEG)

            # ---- persistent activations -----------------------------------
            qT = [pers.tile([128, N], F32R, tag=f"qT{p}", name=f"qT{p}") for p in range(PAIRS)]
            kT = [pers.tile([128, N], F32R, tag=f"kT{p}", name=f"kT{p}") for p in range(PAIRS)]
            v_sb = pers.tile([128, NB, 256], BF16, tag="v")
            outT = [pers.tile([128, N], F32R, tag=f"outT{p}", name=f"outT{p}") for p in range(PAIRS)]
            sAB = ctx.enter_context(contextlib.ExitStack())
            pTpool = sAB.enter_context(tc.tile_pool(name="pTpool", bufs=1))
            pT = pTpool.tile([128, N], F32R, tag="pT")

            # ---- phase A: pT (streamed posT, scalar queue) + q^T/k^T/v ----
            with contextlib.ExitStack() as s2:
                ppa = s2.enter_context(tc.tile_pool(name="ppsum", bufs=1, space="PSUM"))
                pp = s2.enter_context(tc.tile_pool(name="qpsum", bufs=1, space="PSUM"))
                stream = s2.enter_context(tc.tile_pool(name="xstream", bufs=1))
                postream = s2.enter_context(tc.tile_pool(name="postream", bufs=4))
                # scalar-queue loads: wp/bp + streamed posT (ACT idle at start)
                wp_sb = stream.tile([128, KC, D], F32R, tag="wp")
                nc.scalar.dma_start(out=wp_sb, in_=wp[:, :].rearrange("(kc p) m -> p kc m", p=128))
                bp_sb = stream.tile([128, 1], F32, tag="bp")
                nc.scalar.dma_start(out=bp_sb[:D, :], in_=bps[:, :])
                # sync-queue loads: qkv weights + x^T
                wq_sb = stream.tile([128, KC, 256], F32R, tag="wq")
                wk_sb = stream.tile([128, KC, 256], F32R, tag="wk")
                wv_sb = stream.tile([128, KC, 256], F32R, tag="wv")
                nc.sync.dma_start(out=wq_sb, in_=wq[:, :].rearrange("(kc p) m -> p kc m", p=128))
                nc.sync.dma_start(out=wk_sb, in_=wk[:, :].rearrange("(kc p) m -> p kc m", p=128))
                nc.sync.dma_start(out=wv_sb, in_=wv[:, :].rearrange("(kc p) m -> p kc m", p=128))
                bq_sb = stream.tile([128, PAIRS], F32, tag="bq")
                bk_sb = stream.tile([128, PAIRS], F32, tag="bk")
                for p in range(PAIRS):
                    nc.sync.dma_start(out=bq_sb[:, p:p + 1], in_=bq[128 * p:128 * p + 128, :])
                    nc.sync.dma_start(out=bk_sb[:, p:p + 1], in_=bks[128 * p:128 * p + 128, :])
                bvb_sb = stream.tile([128, 256], F32, tag="bvb")
                nc.sync.dma_start(out=bvb_sb, in_=bvb[:, :])
                x_t = []
                for kc in range(KC):
                    t = stream.tile([128, N], F32R, tag=f"xt{kc}")
                    nc.sync.dma_start(out=t, in_=xT[128 * kc:128 * kc + 128, :])
                    x_t.append(t)

                # q^T / k^T, with pT kc-slices interleaved between groups
                pts = [ppa.tile([D, CH], F32, tag=f"ppt{c}", name=f"ppt{c}") for c in range(N // CH)]

                def emit_pt(kcs):
                    for kc in kcs:
                        pos_t = postream.tile([128, N], F32R, tag="pos", name=f"post{kc}")
                        nc.gpsimd.dma_start(out=pos_t, in_=posT[128 * kc:128 * kc + 128, :])
                        for c in range(N // CH):
                            nc.tensor.matmul(
                                pts[c], wp_sb[:, kc, :], pos_t[:, CH * c:CH * c + CH],
                                start=(kc == 0), stop=(kc == KC - 1),
                            )

                pt_sched = {0: [0, 1], 1: [2, 3], 2: [4, 5], 3: [6, 7]}
                gi = 0
                for p in range(PAIRS):
                    for qk in range(2):
                        pss = [pp.tile([128, CH], F32, tag=f"ps{c}", name=f"pqk{qk}_{p}_{c}") for c in range(N // CH)]
                        w_sb = wq_sb if qk == 0 else wk_sb
                        for kc in range(KC):
                            for c in range(N // CH):
                                nc.tensor.matmul(
                                    pss[c], w_sb[:, kc, 128 * p:128 * p + 128],
                                    x_t[kc][:, CH * c:CH * c + CH],
                                    start=(kc == 0), stop=(kc == KC - 1),
                                )
                        for c in range(N // CH):
                            nc.scalar.activation(
                                out=(qT if qk == 0 else kT)[p][:, CH * c:CH * c + CH],
                                in_=pss[c],
                                func=mybir.ActivationFunctionType.Identity,
                                bias=(bq_sb if qk == 0 else bk_sb)[:, p:p + 1],
                                scale=(1.0 if qk == 0 else SCALE),
                            )
                        emit_pt(pt_sched[gi])
                        gi += 1
                for c in range(N // CH):
                    nc.scalar.activation(
                        out=pT[:D, CH * c:CH * c + CH], in_=pts[c],
                        func=mybir.ActivationFunctionType.Identity,
                        bias=bp_sb[:D, :], scale=SCALE,
                    )
                # duplicate p^T into partitions 64:128 for head-pair packing
                nc.sync.dma_start(out=pT[D:2 * D, :], in_=pT[:D, :])
                # v (natural layout), 4 j-blocks per pass to fit 4 psum slots
                for grp in range(4):
                    psvs = [pp.tile([128, 256], F32, tag=f"ps{j}", name=f"psv{grp}_{j}") for j in range(4)]
                    for kc in range(KC):
                        for j in range(4):
                            jb = 4 * grp + j
                            nc.tensor.matmul(
                                psvs[j], x_t[kc][:, 128 * jb:128 * jb + 128],
                                wv_sb[:, kc, :],
                                start=(kc == 0), stop=(kc == KC - 1),
                            )
                    for j in range(4):
                        jb = 4 * grp + j
                        nc.vector.tensor_add(out=v_sb[:, jb, :], in0=psvs[j], in1=bvb_sb)


            # ---- phase B: U_h = q_h . p^T  ->  UB[h] (plain rows) ---------
            with contextlib.ExitStack() as s3:
              if "B" in phases:
                pp = s3.enter_context(tc.tile_pool(name="upsum", bufs=4, space="PSUM"))
                ust = s3.enter_context(tc.tile_pool(name="ustage", bufs=8))
                # pre-zero the spill-read region (rows 1..1536, cols 0..512)
                for h in range(HPC):
                    # neg-fill rows 0..2047 cols [0,128): spill-read mask region
                    dst = bass.AP(
                        tensor=UB,
                        offset=h * N * N,
                        ap=[[128 * N, NB], [N, 128], [1, 128]],
                    )
                    srcb = bass.AP(
                        tensor=neg_sb.tensor,
                        offset=neg_sb.offset,
                        ap=[neg_sb.ap[0], [0, NB], [1, 128]],
                    )
                    nc.sync.dma_start(out=dst, in_=srcb)
                for p in range(PAIRS):
                    for I in range(NB):
                        i0 = 128 * I
                        r0 = N - 128 - i0
                        width = i0 + 128
                        ub2 = ust.tile([128, 2, N], BF16, tag="ub2", name=f"ub2_{p}_{I}")
                        for ci, rc in enumerate(range(r0, N, CH)):
                            w = min(CH, N - rc)
                            pss = [pp.tile([128, CH], F32, tag=f"psu{half}", name=f"psu{half}_{p}_{I}_{ci}")
                                   for half in range(2)]
                            for half in range(2):
                                nc.tensor.matmul(
                                    pss[half][:, :w],
                                    qT[p][D * half:D * half + D, i0:i0 + 128],
                                    pT[D * half:D * half + D, rc:rc + w],
                                    start=True, stop=True,
                                    tile_position=(D * half, 0),
                                )
                            oc = rc - r0
                            for half in range(2):
                                if (ci + half) % 2 == 0:
                                    nc.scalar.activation(
                                        out=ub2[:, half, oc:oc + w], in_=pss[half][:, :w],
                                        func=mybir.ActivationFunctionType.Copy,
                                    )
                                else:
                                    nc.vector.tensor_copy(
                                        out=ub2[:, half, oc:oc + w], in_=pss[half][:, :w]
                                    )
                        dst = bass.AP(
                            tensor=UB,
                            offset=(2 * p) * N * N + i0 * N + r0,
                            ap=[[N, 128], [N * N, 2], [1, width]],
                        )
                        nc.sync.dma_start(out=dst, in_=ub2[:, :, :width])
            sAB.close()  # free pT

            # ---- phase C: scores, softmax, P^T, attn@v --------------------
            with contextlib.ExitStack() as s4:
              if "C" in phases:
                spp = s4.enter_context(tc.tile_pool(name="spsum", bufs=3, space="PSUM"))
                tpp = s4.enter_context(tc.tile_pool(name="tpsum", bufs=2, space="PSUM"))
                ppool = s4.enter_context(tc.tile_pool(name="ppool", bufs=3))
                den_all = [None, None]
                denT = [None, None]
                ptpool = s4.enter_context(tc.tile_pool(name="ptpool", bufs=1))
                pospool = s4.enter_context(tc.tile_pool(name="pospool", bufs=2))
                dpool = s4.enter_context(tc.tile_pool(name="dpool", bufs=8))
                npool = s4.enter_context(tc.tile_pool(name="npool", bufs=1))

                for p in range(PAIRS):
                    den_all[p] = npool.tile([128, NB, 2], F32, tag=f"den{p}", name=f"den{p}")
                    # PT[half][J]: transposed probs [j-part, i-free], bf16
                    PTG = [
                        [
                            ptpool.tile(
                                [128, 4, N - CH * Jg], BF16,
                                tag=f"PTG{half}_{Jg}", name=f"PTG{half}_{Jg}_{p}"
                            )
                            for Jg in range(NB // 4)
                        ]
                        for half in range(2)
                    ]
                    # zero the within-superblock leading i-region of each PT column
                    for half in range(2):
                        for J in range(NB):
                            lead = 128 * J - CH * (J // 4)
                            if lead > 0:
                                nc.vector.memset(PTG[half][J // 4][:, J % 4, :lead], 0.0)

                    for I in range(NB):
                        i0 = 128 * I
                        nchunks = _span_chunks(I)
                        span = i0 + 128
                        last_w = span - CH * (nchunks - 1)
                        P_ts, daccs = [], []
                        pos2 = pospool.tile([128, 2, N], BF16, tag="pos2", name=f"pos2_{p}_{I}")
                        src_ap = bass.AP(
                            tensor=UB,
                            offset=(2 * p) * N * N + i0 * (N - 1) + (N - 1),
                            ap=[[N - 1, 128], [N * N, 2], [1, span]],
                        )
                        nc.sync.dma_start(out=pos2[:, :, :span], in_=src_ap)
                        for half in range(2):
                            P_ts.append(ppool.tile([128, N], BF16, tag=f"P{half}", name=f"P{half}_{p}_{I}"))
                            daccs.append(dpool.tile([128, 4], F32, tag=f"dacc{half}", name=f"dacc{half}_{p}_{I}"))
                        for c in range(nchunks):
                            w = last_w if c == nchunks - 1 else CH
                            pss = [spp.tile([128, CH], F32, tag=f"pss{half}", name=f"pss{half}_{p}_{I}_{c}")
                                   for half in range(2)]
                            for half in range(2):
                                nc.tensor.matmul(
                                    pss[half][:, :w],
                                    qT[p][D * half:D * half + D, i0:i0 + 128],
                                    kT[p][D * half:D * half + D, CH * c:CH * c + w],
                                    start=True, stop=False,
                                    tile_position=(D * half, 0),
                                    skip_group_check=True,
                                )
                            for half in range(2):
                                nc.tensor.matmul(
                                    pss[half][:, :w], ident_sb,
                                    pos2[:, half, CH * c:CH * c + w],
                                    start=False, stop=True,
                                    skip_group_check=True,
                                )
                            for half in range(2):
                                if c == nchunks - 1 and I >= 11:
                                    nc.vector.tensor_add(
                                        out=pss[half][:, :w], in0=pss[half][:, :w],
                                        in1=masks_sb[:, I % 4, :w],
                                    )
                                nc.scalar.activation(
                                    out=P_ts[half][:, CH * c:CH * c + w], in_=pss[half][:, :w],
                                    func=mybir.ActivationFunctionType.Exp,
                                    accum_out=daccs[half][:, c:c + 1],
                                )
                        for half in range(2):
                            P_t = P_ts[half]
                            dacc = daccs[half]
                            # denominator -> den_all[p][:, I, half]
                            if nchunks > 1:
                                nc.vector.reduce_sum(
                                    out=den_all[p][:, I, half:half + 1],
                                    in_=dacc[:, :nchunks],
                                    axis=mybir.AxisListType.X,
                                )
                            else:
                                nc.vector.tensor_copy(
                                    out=den_all[p][:, I, half:half + 1],
                                    in_=dacc[:, :1],
                                )
                            # transposes into PTG[half][Jg][:, J%4, i0:i0+128]
                            for Jg in range((I + 4) // 4):
                                nj = min(4, I + 1 - 4 * Jg)
                                pstw = tpp.tile([128, 4, 128], BF16, tag="pst")
                                for t in range(nj):
                                    J = 4 * Jg + t
                                    nc.tensor.transpose(
                                        pstw[:, t, :], P_t[:, 128 * J:128 * J + 128],
                                        ident_sb,
                                    )
                                off = i0 - CH * Jg
                                if (I + Jg) % 2 == 0:
                                    nc.scalar.activation(
                                        out=PTG[half][Jg][:, :nj, off:off + 128],
                                        in_=pstw[:, :nj, :],
                                        func=mybir.ActivationFunctionType.Copy,
                                    )
                                else:
                                    nc.vector.tensor_copy(
                                        out=PTG[half][Jg][:, :nj, off:off + 128],
                                        in_=pstw[:, :nj, :],
                                    )
                        # attn @ v per completed 512-superblock
                        if I % 4 == 3:
                            s = I // 4
                            psav = spp.tile([128, CH], F32, tag=f"pss{s % 2}", name=f"psav_{p}_{s}")
                            for J in range(4 * s + 4):
                                ioff = CH * s - CH * (J // 4)
                                for half in range(2):
                                    nc.tensor.matmul(
                                        psav[D * half:D * half + D, :],
                                        v_sb[:, J, 64 * (2 * p + half):64 * (2 * p + half) + D],
                                        PTG[half][J // 4][:, J % 4, ioff:ioff + CH],
                                        start=(J == 0), stop=(J == 4 * s + 3),
                                        tile_position=(0, D * half),
                                        skip_group_check=True,
                                    )
                            nc.scalar.activation(
                                out=outT[p][:, CH * s:CH * s + CH], in_=psav,
                                func=mybir.ActivationFunctionType.Copy,
                            )
                    # ---- post-AV normalization for this pair ----
                    denT[p] = npool.tile([2, N], F32, tag="denT", name=f"denT{p}")
                    for I in range(NB):
                        pst2 = tpp.tile([128, 128], F32, tag="pst")
                        nc.tensor.transpose(
                            pst2[:2, :], den_all[p][:, I, :], ident32_sb
                        )
                        nc.scalar.activation(
                            out=denT[p][:, 128 * I:128 * I + 128], in_=pst2[:2, :128],
                            func=mybir.ActivationFunctionType.Copy,
                        )
                    denTr = npool.tile([2, N], F32R, tag="denTr", name=f"denTr{p}")
                    with nc.allow_low_precision(reason="f32r view for PE broadcast"):
                        nc.vector.reciprocal(out=denTr, in_=denT[p])
                    rb = npool.tile([128, N], F32, tag="rb", name=f"rb{p}")
                    for c in range(N // CH):
                        psb = spp.tile([128, CH], F32, tag=f"pss{c % 2}", name=f"psb_{p}_{c}")
                        nc.tensor.matmul(
                            psb, pattern2_sb, denTr[:, CH * c:CH * c + CH],
                            start=True, stop=True,
                        )
                        nc.vector.tensor_copy(out=rb[:, CH * c:CH * c + CH], in_=psb)
                    nc.vector.tensor_mul(out=outT[p], in0=outT[p], in1=rb)

            # ---- phase D: out partial = outT^T @ Wo_rows ------------------
            with contextlib.ExitStack() as s5:
              if "D" in phases:
                opp = s5.enter_context(tc.tile_pool(name="opsum", bufs=3, space="PSUM"))
                ost = s5.enter_context(tc.tile_pool(name="ostage", bufs=4))
                wo_sb = [ost.tile([128, DIM], F32R, tag=f"wo{p}", name=f"wo{p}") for p in range(PAIRS)]
                for p in range(PAIRS):
                    nc.sync.dma_start(out=wo_sb[p], in_=wo[128 * p:128 * p + 128, :])
                for Ip in range(NB // 2):
                    o2 = ost.tile([128, 2, DIM], BF16, tag="o2", name=f"o2_{Ip}")
                    for b2 in range(2):
                        I = 2 * Ip + b2
                        i0 = 128 * I
                        pso = opp.tile([128, DIM], F32, tag="pso", name=f"pso_{I}")
                        for c in range(DIM // CH):
                            for p in range(PAIRS):
                                nc.tensor.matmul(
                                    pso[:, CH * c:CH * c + CH],
                                    outT[p][:, i0:i0 + 128],
                                    wo_sb[p][:, CH * c:CH * c + CH],
                                    start=(p == 0), stop=(p == PAIRS - 1),
                                    skip_group_check=True,
                                )
                        if b2 == 0:
                            nc.vector.tensor_copy(out=o2[:, b2, :], in_=pso)
                        else:
                            nc.scalar.activation(
                                out=o2[:, b2, :], in_=pso,
                                func=mybir.ActivationFunctionType.Copy,
                            )
                    dst = bass.AP(
                        tensor=out,
                        offset=256 * Ip * DIM,
                        ap=[[DIM, 128], [128 * DIM, 2], [1, DIM]],
                    )
                    nc.sync.dma_start(out=dst, in_=o2)

    _patch_bass(nc)
    return nc


_NC_CACHE = {}


def _get_nc():
    if "nc" not in _NC_CACHE:
        _NC_CACHE["nc"] = build_nc()
    return _NC_CACHE["nc"]


def kernel(x, pos_emb, Wq, bq, Wkv, bkv, Wp, bp, Wo, bo):
    x = np.asarray(x, dtype=np.float32)
    pos_emb = np.asarray(pos_emb, dtype=np.float32)
    Wq = np.asarray(Wq, dtype=np.float32)
    bq = np.asarray(bq, dtype=np.float32)
    Wkv = np.asarray(Wkv, dtype=np.float32)
    bkv = np.asarray(bkv, dtype=np.float32)
    Wp = np.asarray(Wp, dtype=np.float32)
    bp = np.asarray(bp, dtype=np.float32)
    Wo = np.asarray(Wo, dtype=np.float32)
    bo = np.asarray(bo, dtype=np.float32)

    b, n, dim = x.shape
    assert (b, n, dim) == (2, N, DIM)

    xTs = [np.ascontiguousarray(x[bi].T) for bi in range(b)]
    posT = np.ascontiguousarray(pos_emb.T)

    ident = np.eye(128, dtype=np.float16)
    masks = np.zeros((128, 4, CH), dtype=np.float16)
    for v in range(4):
        for a in range(128):
            masks[a, v, 128 * v + a + 1:] = NEG

    in_maps = []
    for c in range(8):
        bi, g = divmod(c, HPC)
        cols = slice(256 * g, 256 * g + 256)
        in_maps.append(
            {
                "xT": xTs[bi],
                "posT": posT,
                "wq": np.ascontiguousarray(Wq[:, cols]),
                "wk": np.ascontiguousarray(Wkv[:, 256 * g:256 * g + 256]),
                "wv": np.ascontiguousarray(Wkv[:, DIM + 256 * g:DIM + 256 * g + 256]),
                "wp": Wp,
                "wo": np.ascontiguousarray(Wo[256 * g:256 * g + 256, :]),
                "bq": np.ascontiguousarray(bq[cols])[:, None],
                "bks": (np.ascontiguousarray(bkv[256 * g:256 * g + 256]) * SCALE)[:, None],
                "bvb": np.broadcast_to(
                    bkv[DIM + 256 * g:DIM + 256 * g + 256], (128, 256)
                ).copy(),
                "bps": (bp * SCALE)[:, None],
                "ident": ident,
                "ident32": np.eye(128, dtype=np.float32),
                "pattern2": np.repeat(np.eye(2, dtype=np.float32), 64, axis=1),
                "masks": masks,
            }
        )

    nc = _get_nc()
    res = run_bass_kernel_spmd(nc, in_maps, core_ids=list(range(8)))

    outp = np.zeros((b, n, dim), dtype=np.float32)
    for c in range(8):
        bi = c // HPC
        outp[bi] += res.results[c]["out"].astype(np.float32)
    outp += bo
    return outp

